# revision 1
# baseline (speedup 1.0000x reference)
"""GCN layer kernel for Trainium2: out[b] = D^-1/2 (A[b]+I) D^-1/2 H[b] B.

Data-parallel, one graph per NeuronCore, no collectives.

Refactoring (never materializes the normalized adjacency):
    P = H @ B;  X = d ⊙rows P;  Y = A @ X + X;  out = d ⊙rows Y
with d = 1/sqrt(1 + rowsum(A)).

Device works in transposed space: host passes AT = A[b].T, HT = H[b].T (pure
layout prep), PE computes YT = X^T @ A^T (+ X^T via identity matmul)
contracting over SBUF partitions, the epilogue scales YT columns by a
broadcast d built from a PE outer product, and the host transposes the
[O, N] result back.

deg (= rowsum A = colsum AT) comes from ones-weight matmuls overlapping the
AT DMA stream; chunk sizes taper (4,4,4,2,1,1 slabs) so the final chunk's
deg matmuls add only ~2us after the last DMA byte. All matmuls are float32r
(full fp32 element precision at 1 cycle/row, verified on HW). rsqrt runs
per-128-column on a transposed [128,1] layout so the first X tile is ready
~1us after deg completes.
"""
import sys

sys.path.insert(0, "/opt/trn_rl_repo")

import numpy as np

B_, N_, F_, O_ = 8, 2048, 128, 128
NT = N_ // 128  # 16 slabs of AT
CHUNKS = [4, 4, 4, 2, 1, 1]  # slabs per DMA chunk (tapered tail)
N_CORES = 8

_CACHE = {}
LAST_RESULTS = None


def _build_program():
    import concourse.bacc as bacc
    import concourse.tile as tile
    import concourse.mybir as mybir

    f32 = mybir.dt.float32
    f32r = mybir.dt.float32r
    AF = mybir.ActivationFunctionType

    nc = bacc.Bacc(None, target_bir_lowering=False)
    AT = nc.dram_tensor("at", [N_, N_], f32r, kind="ExternalInput")
    HT = nc.dram_tensor("ht", [F_, N_], f32r, kind="ExternalInput")
    # consts: [bw | eye | ones | ones_col]
    CST = nc.dram_tensor("consts", [128, 385], f32r, kind="ExternalInput")
    OT = nc.dram_tensor("ot", [O_, N_], f32, kind="ExternalOutput")

    at_view = AT.rearrange("(s p) i -> p s i", p=128)  # [128, NT, N_]

    chunk_start = []
    s0 = 0
    for csz in CHUNKS:
        chunk_start.append(s0)
        s0 += csz

    with tile.TileContext(nc) as tc:
        with (
            tc.tile_pool(name="const", bufs=1) as cst,
            tc.tile_pool(name="achunks", bufs=1) as ach,
            tc.tile_pool(name="small", bufs=1) as sml,
            tc.tile_pool(name="outp", bufs=2) as outp,
            tc.tile_pool(name="psbig", bufs=1, space="PSUM") as psb,
            tc.tile_pool(name="pssmall", bufs=3, space="PSUM") as pss,
        ):
            cst_sb = cst.tile([128, 385], f32r, tag="cst")
            ht_sb = cst.tile([128, N_], f32r, tag="ht")
            nc.sync.dma_start(out=cst_sb, in_=CST[:, :])
            nc.sync.dma_start(out=ht_sb, in_=HT[:, :])
            bw = cst_sb[:, 0:128]
            eye = cst_sb[:, 128:256]
            ones = cst_sb[:, 256:384]
            onesf = cst_sb[:, 384:385].bitcast(f32)
            eyef = cst_sb[:, 128:256].bitcast(f32)

            # A^T resident chunks; all DMAs issued up-front (FIFO on SP ring)
            at_slab = [None] * NT
            for ci, csz in enumerate(CHUNKS):
                st = chunk_start[ci]
                t = ach.tile([128, csz, N_], f32r, tag=f"at{ci}")
                nc.sync.dma_start(out=t, in_=at_view[:, st : st + csz, :])
                for sl in range(csz):
                    at_slab[st + sl] = t[:, sl, :]

            # P = H @ B, evacuated to SBUF unscaled (fp32)
            p_sb = sml.tile([128, NT, O_], f32, tag="p")
            for t in range(NT):
                p_ps = pss.tile([128, O_], f32, tag="sm")
                nc.tensor.matmul(
                    p_ps, ht_sb[:, t * 128 : (t + 1) * 128], bw, start=True, stop=True
                )
                nc.vector.tensor_copy(p_sb[:, t, :], p_ps)

            # deg broadcast: ones.T @ AT accumulated over slabs, overlaps DMA
            deg_ps = psb.tile([128, N_], f32, tag="big")
            for s in range(NT):
                for ib in range(4):
                    nc.tensor.matmul(
                        deg_ps[:, ib * 512 : (ib + 1) * 512],
                        ones,
                        at_slab[s][:, ib * 512 : (ib + 1) * 512],
                        start=(s == 0),
                        stop=(s == NT - 1),
                    )

            # d-chain, pipelined per 512-chunk of deg: sqrt on ACT row 0,
            # PE-transpose each 128-chunk to [128,1], reciprocal per column,
            # and scale that column's X tile immediately.
            dgsq_sb = sml.tile([1, N_], f32, tag="dgsq")
            d_sb = sml.tile([128, NT], f32, tag="d")
            xs = []
            for t in range(NT):
                x_t = sml.tile([128, O_], f32r, tag=f"x{t}")
                xs.append(x_t)
            for q in range(4):
                nc.scalar.activation(
                    out=dgsq_sb[:, q * 512 : (q + 1) * 512],
                    in_=deg_ps[0:1, q * 512 : (q + 1) * 512],
                    func=AF.Sqrt,
                    bias=1.0,
                    scale=1.0,
                )
                for t in range(q * 4, q * 4 + 4):
                    tp_ps = pss.tile([128, 1], f32, tag="sm")
                    nc.tensor.transpose(
                        tp_ps, dgsq_sb[0:1, t * 128 : (t + 1) * 128], onesf[0:1, 0:1]
                    )
                    nc.vector.tensor_copy(d_sb[:, t : t + 1], tp_ps)
                    nc.vector.reciprocal(
                        out=d_sb[:, t : t + 1], in_=d_sb[:, t : t + 1]
                    )
                    nc.vector.tensor_scalar_mul(
                        xs[t], p_sb[:, t, :], d_sb[:, t : t + 1]
                    )

            # broadcast d over partitions: transpose d_sb -> [16,128], flatten
            # to a [1, 2048] row via a tiny SWDGE DMA (16x512B descriptors),
            # then 4 outer-product matmuls ones[1,128]^T @ d_row -> [128,512]
            dT_ps = pss.tile([16, 128], f32, tag="sm")
            nc.tensor.transpose(dT_ps, d_sb, eyef)
            dT_sb = sml.tile([16, 128], f32, tag="dT")
            nc.vector.tensor_copy(dT_sb, dT_ps)
            d_row = sml.tile([1, N_], f32r, tag="drow")
            nc.gpsimd.dma_start(
                out=d_row[0:1, :].rearrange("a (t p) -> a t p", t=16),
                in_=dT_sb[:, :],
            )

            yt_ps = psb.tile([128, N_], f32, tag="big")
            dbc_sb = sml.tile([128, N_], f32, tag="dbc")

            def emit_mms(ib):
                blk = slice(ib * 512, (ib + 1) * 512)
                for t in range(NT):
                    nc.tensor.matmul(
                        yt_ps[:, blk],
                        xs[t],
                        at_slab[t][:, ib * 512 : (ib + 1) * 512],
                        start=(t == 0),
                        stop=False,
                    )
                for c in range(4):
                    cc = ib * 4 + c
                    nc.tensor.matmul(
                        yt_ps[:, cc * 128 : (cc + 1) * 128],
                        xs[cc],
                        eye,
                        start=False,
                        stop=(c == 3),
                    )

            def emit_outer():
                for q in range(4):
                    obc_ps = pss.tile([128, 512], f32, tag="sm")
                    nc.tensor.matmul(
                        obc_ps,
                        ones[0:1, 0:128],
                        d_row[0:1, q * 512 : (q + 1) * 512],
                        start=True,
                        stop=True,
                    )
                    nc.vector.tensor_copy(dbc_sb[:, q * 512 : (q + 1) * 512], obc_ps)

            def emit_tail(ib):
                blk = slice(ib * 512, (ib + 1) * 512)
                ost = outp.tile([128, 512], f32, tag="ost")
                nc.vector.tensor_mul(ost, yt_ps[:, blk], dbc_sb[:, blk])
                nc.sync.dma_start(out=OT[:, blk], in_=ost)

            for ib in range(4):
                emit_mms(ib)
                if ib == 0:
                    emit_outer()
                else:
                    emit_tail(ib - 1)
            emit_tail(3)

    nc.compile()
    return nc


def _get_program():
    if "nc" not in _CACHE:
        _CACHE["nc"] = _build_program()
    return _CACHE["nc"]


def _make_consts():
    c = np.zeros((128, 385), dtype=np.float32)
    c[:, 128:256] = np.eye(128, dtype=np.float32)
    c[:, 256:384] = 1.0
    c[:, 384] = 1.0
    return c


def kernel(H, A, B):
    global LAST_RESULTS
    from concourse.bass_utils import run_bass_kernel_spmd

    nc = _get_program()
    consts = _make_consts()

    in_maps = []
    for b in range(B_):
        cst = consts.copy()
        cst[:, 0:128] = np.asarray(B, dtype=np.float32)
        in_maps.append(
            {
                "at": np.ascontiguousarray(np.asarray(A[b], dtype=np.float32).T),
                "ht": np.ascontiguousarray(np.asarray(H[b], dtype=np.float32).T),
                "consts": cst,
            }
        )

    res = run_bass_kernel_spmd(nc, in_maps, list(range(N_CORES)))
    LAST_RESULTS = res

    out = np.empty((B_, N_, O_), dtype=np.float32)
    for b in range(B_):
        out[b] = res.results[b]["ot"].T
    return out



# revision 2
# speedup vs baseline: 1.2554x; 1.2554x over previous
"""GCN layer kernel for Trainium2: out[b] = D^-1/2 (A[b]+I) D^-1/2 H[b] B.

Data-parallel, one graph per NeuronCore, no collectives.

v2 (bf16 streaming): host ships AT1 = (A[b]+I).T and HT = H[b].T in bf16
(halves HBM traffic vs fp32; rel err ~3e-3, well under the 2e-2 gate, and
bf16 enables FWL so weight loads hide under matmuls, unlike fp32-HIGH).

Refactoring (never materializes the normalized adjacency):
    dsq = sqrt(rowsum(A1));  xht = HT / dsq (free-dim bcast);
    X_t = xht_t^T @ B;  YT = sum_t X_t^T-style accum: yt += X_t^T A1T_t;
    out^T = yt / dsq.
Folding d into HT *before* the P matmul (instead of scaling P rows) avoids
every partition-transpose of d: the scale index is a free dim of HT, so one
tensor_tensor multiply against the all-partitions-equal rsqrt row does it.

deg comes from ones^T @ AT1 matmuls that track the chunked DMA stream; the
serial tail after the last A byte is just ACT sqrt -> DVE recip -> DVE mul
-> 16 (P' + Y) matmul pairs per output block. A+I is folded in on the host
so no identity matmuls are needed.
"""
import sys

sys.path.insert(0, "/opt/trn_rl_repo")

import numpy as np

B_, N_, F_, O_ = 8, 2048, 128, 128
NT = N_ // 128  # 16 slabs of AT
CHUNKS = [2, 2, 2, 2, 2, 2, 2, 1, 1]  # slabs per DMA chunk (tapered tail)
N_CORES = 8

_CACHE = {}
LAST_RESULTS = None


def _build_program():
    import concourse.bacc as bacc
    import concourse.tile as tile
    import concourse.mybir as mybir

    f32 = mybir.dt.float32
    bf16 = mybir.dt.bfloat16
    AF = mybir.ActivationFunctionType

    nc = bacc.Bacc(None, target_bir_lowering=False)
    AT = nc.dram_tensor("at", [N_, N_], bf16, kind="ExternalInput")
    HT = nc.dram_tensor("ht", [F_, N_], bf16, kind="ExternalInput")
    # consts: [bw | ones]
    CST = nc.dram_tensor("consts", [128, 256], bf16, kind="ExternalInput")
    OT = nc.dram_tensor("ot", [O_, N_], f32, kind="ExternalOutput")

    at_view = AT.rearrange("(s p) i -> p s i", p=128)  # [128, NT, N_]

    chunk_start = []
    s0 = 0
    for csz in CHUNKS:
        chunk_start.append(s0)
        s0 += csz

    with tile.TileContext(nc) as tc:
        with (
            tc.tile_pool(name="const", bufs=1) as cst,
            tc.tile_pool(name="achunks", bufs=1) as ach,
            tc.tile_pool(name="small", bufs=1) as sml,
            tc.tile_pool(name="outp", bufs=2) as outp,
            tc.tile_pool(name="psbig", bufs=1, space="PSUM") as psb,
            tc.tile_pool(name="pssmall", bufs=3, space="PSUM") as pss,
        ):
            cst_sb = cst.tile([128, 256], bf16, tag="cst")
            ht_sb = cst.tile([128, N_], bf16, tag="ht")
            # consts + ht on the ACT HWDGE ring so their descriptor-gen
            # overlaps the big AT stream on the SP ring
            nc.scalar.dma_start(out=cst_sb, in_=CST[:, :])
            nc.scalar.dma_start(out=ht_sb, in_=HT[:, :])
            bw = cst_sb[:, 0:128]
            ones = cst_sb[:, 128:256]

            # A^T resident chunks; all DMAs issued up-front (FIFO on SP ring)
            at_slab = [None] * NT
            for ci, csz in enumerate(CHUNKS):
                st = chunk_start[ci]
                t = ach.tile([128, csz, N_], bf16, tag=f"at{ci}")
                nc.sync.dma_start(out=t, in_=at_view[:, st : st + csz, :])
                for sl in range(csz):
                    at_slab[st + sl] = t[:, sl, :]

            # deg broadcast: ones^T @ AT accumulated over slabs, overlaps DMA
            deg_ps = psb.tile([128, N_], f32, tag="big")
            for s in range(NT):
                for ib in range(4):
                    nc.tensor.matmul(
                        deg_ps[:, ib * 512 : (ib + 1) * 512],
                        ones,
                        at_slab[s][:, ib * 512 : (ib + 1) * 512],
                        start=(s == 0),
                        stop=(s == NT - 1),
                    )

            # d-chain per 512-chunk: dsq = sqrt(deg) on ACT (PSUM read, all
            # partitions equal), dbc = 1/dsq on DVE, xht = ht * dbc (bf16).
            dsq_sb = sml.tile([128, N_], f32, tag="dsq")
            dbc_sb = sml.tile([128, N_], f32, tag="dbc")
            xht_sb = sml.tile([128, N_], bf16, tag="xht")
            for q in range(4):
                blk = slice(q * 512, (q + 1) * 512)
                nc.scalar.activation(
                    out=dsq_sb[:, blk], in_=deg_ps[:, blk], func=AF.Sqrt
                )
                nc.vector.reciprocal(out=dbc_sb[:, blk], in_=dsq_sb[:, blk])
                nc.vector.tensor_mul(xht_sb[:, blk], ht_sb[:, blk], dbc_sb[:, blk])

            yt_ps = psb.tile([128, N_], f32, tag="big")

            # X_t = xht_t^T @ B (PSUM->SBUF bf16), interleaved with the
            # ib=0 Y matmuls so Y starts as soon as X_0 exists
            xs = []
            for t in range(NT):
                x_ps = pss.tile([128, O_], f32, tag="sm")
                nc.tensor.matmul(
                    x_ps, xht_sb[:, t * 128 : (t + 1) * 128], bw, start=True, stop=True
                )
                x_sb = sml.tile([128, O_], bf16, tag=f"x{t}")
                nc.vector.tensor_copy(x_sb, x_ps)
                xs.append(x_sb)
                nc.tensor.matmul(
                    yt_ps[:, 0:512],
                    x_sb,
                    at_slab[t][:, 0:512],
                    start=(t == 0),
                    stop=(t == NT - 1),
                )

            def emit_mms(ib):
                blk = slice(ib * 512, (ib + 1) * 512)
                for t in range(NT):
                    nc.tensor.matmul(
                        yt_ps[:, blk],
                        xs[t],
                        at_slab[t][:, blk],
                        start=(t == 0),
                        stop=(t == NT - 1),
                    )

            def emit_tail(ib):
                blk = slice(ib * 512, (ib + 1) * 512)
                ost = outp.tile([128, 512], f32, tag="ost")
                nc.vector.tensor_mul(ost, yt_ps[:, blk], dbc_sb[:, blk])
                nc.sync.dma_start(out=OT[:, blk], in_=ost)

            for ib in range(1, 4):
                emit_mms(ib)
                emit_tail(ib - 1)
            emit_tail(3)

    nc.compile()
    return nc


def _get_program():
    if "nc" not in _CACHE:
        _CACHE["nc"] = _build_program()
    return _CACHE["nc"]


def kernel(H, A, B):
    global LAST_RESULTS
    import ml_dtypes
    from concourse.bass_utils import run_bass_kernel_spmd

    nc = _get_program()
    bf16 = ml_dtypes.bfloat16

    consts = np.zeros((128, 256), dtype=bf16)
    consts[:, 0:128] = np.asarray(B, dtype=np.float32).astype(bf16)
    consts[:, 128:256] = np.ones((128, 128), dtype=bf16)

    eye = np.eye(N_, dtype=np.float32)
    in_maps = []
    for b in range(B_):
        a1t = (np.asarray(A[b], dtype=np.float32) + eye).T
        in_maps.append(
            {
                "at": np.ascontiguousarray(a1t).astype(bf16),
                "ht": np.ascontiguousarray(
                    np.asarray(H[b], dtype=np.float32).T
                ).astype(bf16),
                "consts": consts,
            }
        )

    res = run_bass_kernel_spmd(nc, in_maps, list(range(N_CORES)))
    LAST_RESULTS = res

    out = np.empty((B_, N_, O_), dtype=np.float32)
    for b in range(B_):
        out[b] = res.results[b]["ot"].T
    return out
